# revision 1
# baseline (speedup 1.0000x reference)
"""CenterLoss Trainium2 kernel.

Full inputs:
  ep_mask_embed    (8, 4096, 256) f32
  ep_mask          (8, 1, 1024, 1024) f32
  query_mask_embed (8, 4096, 256) f32
  query_mask       (8, 1, 1024, 1024) f32
Output: (3,) f32 = [mean(center_loss), mean(pos_loss), mean(neg_loss)]

Sharding: data-parallel, one batch sample per NeuronCore (8 cores).

Math (per sample, c=256, N=4096, m = mask downsampled to (N,)):
  Everything reduces to three PSUM-accumulated bf16 matmul chains
  (lhsT = [m, 1-m] mask columns, token-on-partition):
    psum_ew  [2,257] += [ep_m,1-ep_m]^T @ [ep_embed | 1]
    psum_qw  [2,257] += [q_m, 1-q_m]^T @ [q_embed  | 1]
    psum_qsw [2,256] += [q_m, 1-q_m]^T @ (q_embed^2)
  followed by a tiny fp32 epilogue (s = rowsum(psum_qsw)):
    pc  = epw_pos/(n_pos_ep+0.1),  ncen = epw_neg/(n_neg_ep+0.1)
    pos_num = s_pos - 2*dot(pc,qw_pos) + n_pos_q*dot(pc,pc)
    pos_loss = pos_num / (max(n_pos_q,1)*c) * min(n_pos_q,1);  neg analogous.
  bf16 is safe: the weighted sums are normalized by n~2048 downstream and
  the s sums average 512K rounding errors (~1e-6 rel on the loss).

Implementation notes:
  - Tokens stream in chunks of 512 rows staged as [128, 4, 256]
    (4 consecutive token rows per partition -> one 4KB contiguous DMA
    descriptor per partition).  Descriptor count is what bounds both the
    HWDGE descriptor-generation time on the sync sequencer (the serial
    bottleneck at 1KB descriptors) and the SDMA queue efficiency.
  - Each chunk runs 4 matmuls per chain, one per token parity g
    (partition p holds tokens 512i+4p+g); the mask weight columns are
    host-permuted to match (pure indexing), and 1-m / counts / bf16
    casts are computed on device at prep time.
  - f32->bf16 converts and the squares are split across ACT and DVE;
    count-derived epilogue scalars are hoisted under the stream.
The host only shards, downsamples (stride-16 indexing), and permutes the
mask values per sample, and means the 8 per-core [pos, neg] pairs.
"""

import numpy as np
from contextlib import ExitStack

import concourse.bass as bass
import concourse.bacc as bacc
import concourse.tile as tile
from concourse import mybir
from concourse.bass_utils import run_bass_kernel_spmd

F32 = mybir.dt.float32
BF16 = mybir.dt.bfloat16

P = 128          # partitions
N_TOK = 4096     # tokens per sample (64*64 patches)
C = 256          # channels
T = 4            # token rows per partition per chunk
DC = P * T       # tokens per chunk (512)
N_DC = N_TOK // DC   # 8 chunks
B = 8            # batch == n cores
PATCH = 16

_CACHE = {}


def _build():
    """Build the per-core Bass program (identical on all cores)."""
    nc = bacc.Bacc("TRN2", target_bir_lowering=False, debug=False)

    ep_embed = nc.dram_tensor("ep_embed", [N_TOK, C], F32, kind="ExternalInput").ap()
    q_embed = nc.dram_tensor("q_embed", [N_TOK, C], F32, kind="ExternalInput").ap()
    # downsampled mask values, host-permuted to the weight-column layout:
    # lm[p, 4i+g] = mask_ds[512i + 4p + g] (pure indexing on host);
    # cols 0:32 = ep mask, cols 32:64 = q mask
    lm = nc.dram_tensor("lm", [P, 2 * N_DC * T], F32, kind="ExternalInput").ap()
    # [pos_loss; neg_loss] on partitions 0/1
    out2 = nc.dram_tensor("out2", [2, 1], F32, kind="ExternalOutput").ap()

    AF = mybir.ActivationFunctionType
    OP = mybir.AluOpType

    with tile.TileContext(nc) as tc, ExitStack() as ctx:
        const_pool = ctx.enter_context(tc.tile_pool(name="const", bufs=1))
        ep_pool = ctx.enter_context(tc.tile_pool(name="ep_pool", bufs=10))
        q_pool = ctx.enter_context(tc.tile_pool(name="q_pool", bufs=10))
        psum_pool = ctx.enter_context(
            tc.tile_pool(name="psum", bufs=1, space=bass.MemorySpace.PSUM)
        )
        fin_pool = ctx.enter_context(tc.tile_pool(name="fin", bufs=1))

        # ---- mask prep: L64 [128, 64] = [m cols (32) | 1-m cols (32)];
        # lhsT for (chunk i, parity g) = cols {4i+g, 4i+g+32} (stride 32) ----
        NM = N_DC * T  # 32 mask columns
        L = {}
        lm_t = const_pool.tile([P, 2 * NM], F32, name="lm_t", tag="lm_t")
        nc.sync.dma_start(out=lm_t[:], in_=lm[:])
        for li, nm in enumerate(("ep", "q")):
            L64 = const_pool.tile([P, 2 * NM], F32, name=f"L64_{nm}", tag=f"L64_{nm}")
            nc.vector.tensor_copy(L64[:, 0:NM], lm_t[:, li * NM:(li + 1) * NM])
            nc.vector.tensor_scalar(
                out=L64[:, NM:2 * NM], in0=L64[:, 0:NM], scalar1=-1.0,
                scalar2=1.0, op0=OP.mult, op1=OP.add,
            )
            Lb = const_pool.tile([P, 2 * NM], BF16, name=f"Lb_{nm}", tag=f"Lb_{nm}")
            nc.vector.tensor_copy(Lb[:], L64[:])
            L[nm] = Lb
            # per-partition mask sums -> [pos, neg] counts via a tiny matmul
            rs = const_pool.tile([P, 2], F32, name=f"rs_{nm}", tag=f"rs_{nm}")
            nc.vector.tensor_reduce(
                rs[:, 0:1], L64[:, 0:NM], axis=mybir.AxisListType.X, op=OP.add)
            nc.vector.tensor_reduce(
                rs[:, 1:2], L64[:, NM:2 * NM], axis=mybir.AxisListType.X,
                op=OP.add)
            L[nm + "_rs"] = rs

        ones1 = const_pool.tile([P, 1], F32, name="ones1", tag="ones1")
        nc.vector.memset(ones1[:], 1.0)

        def lhsT(nm, i, g):
            # 2-column AP [m, 1-m] with free stride NM
            return L[nm].rearrange("p (h c) -> p c h", h=2)[:, T * i + g, :]

        # PSUM accumulators (pos on partition 0, neg on partition 1):
        psum_ew = psum_pool.tile([2, C], F32, name="psum_ew", tag="pew")
        psum_qw = psum_pool.tile([2, C], F32, name="psum_qw", tag="pqw")
        psum_qsw = psum_pool.tile([2, C], F32, name="psum_qsw", tag="pqsw")
        psum_en = psum_pool.tile([2, 1], F32, name="psum_en", tag="pen")
        psum_qn = psum_pool.tile([2, 1], F32, name="psum_qn", tag="pqn")
        nc.tensor.matmul(psum_en[:], L["ep_rs"][:], ones1[:])
        nc.tensor.matmul(psum_qn[:], L["q_rs"][:], ones1[:])

        # count-derived epilogue scalars, hidden under the stream
        ncol = fin_pool.tile([2, 1], F32, name="ncol", tag="ncol")
        nc.vector.tensor_copy(ncol[:], psum_qn[:, 0:1])
        den = fin_pool.tile([2, 1], F32, name="den", tag="den")
        nc.vector.tensor_scalar_add(den[:], psum_en[:, 0:1], 0.1)
        rden = fin_pool.tile([2, 1], F32, name="rden", tag="rden")
        nc.vector.reciprocal(rden[:], den[:])
        nmax = fin_pool.tile([2, 1], F32, name="nmax", tag="nmax")
        nc.vector.tensor_scalar_max(nmax[:], ncol[:], 1.0)
        rn = fin_pool.tile([2, 1], F32, name="rn", tag="rn")
        nc.vector.reciprocal(rn[:], nmax[:])
        gate = fin_pool.tile([2, 1], F32, name="gate", tag="gate")
        nc.vector.tensor_scalar_min(gate[:], ncol[:], 1.0)

        # ---- main streaming loop over 8 chunks of 512 tokens ----
        for i in range(N_DC):
            first, last = i == 0, i == N_DC - 1

            # [128, 4, 256] staging: partition p block g holds token
            # 512i + 4p + g -> one 4KB descriptor per partition
            te = ep_pool.tile([P, T * C], F32, name="te", tag="te")
            src_ = ep_embed[i * DC:(i + 1) * DC, :].rearrange(
                "(p t) c -> p (t c)", t=T)
            nc.sync.dma_start(out=te[:], in_=src_)
            re_bf = ep_pool.tile([P, T * C], BF16, name="re_bf", tag="re_bf")
            nc.vector.tensor_copy(re_bf[:], te[:])
            for g in range(T):
                nc.tensor.matmul(
                    psum_ew[:], lhsT("ep", i, g),
                    re_bf[:, g * C:(g + 1) * C],
                    start=first and g == 0, stop=last and g == T - 1,
                )

            tq = q_pool.tile([P, T * C], F32, name="tq", tag="tq")
            srcq = q_embed[i * DC:(i + 1) * DC, :].rearrange(
                "(p t) c -> p (t c)", t=T)
            nc.sync.dma_start(out=tq[:], in_=srcq)
            rq_bf = q_pool.tile([P, T * C], BF16, name="rq_bf", tag="rq_bf")
            nc.scalar.copy(rq_bf[:], tq[:])
            for g in range(T):
                nc.tensor.matmul(
                    psum_qw[:], lhsT("q", i, g),
                    rq_bf[:, g * C:(g + 1) * C],
                    start=first and g == 0, stop=last and g == T - 1,
                )

            # squares: first half on ACT, second half on DVE
            sq_bf = q_pool.tile([P, T * C], BF16, name="sq_bf", tag="sq_bf")
            h = T * C // 2
            nc.scalar.activation(
                out=sq_bf[:, 0:h], in_=tq[:, 0:h], func=AF.Square)
            nc.vector.tensor_mul(
                sq_bf[:, h:T * C], tq[:, h:T * C], tq[:, h:T * C])
            for g in range(T):
                nc.tensor.matmul(
                    psum_qsw[:], lhsT("q", i, g),
                    sq_bf[:, g * C:(g + 1) * C],
                    start=first and g == 0, stop=last and g == T - 1,
                )

        # ---- epilogue: partition 0 = pos, partition 1 = neg ----
        # (single-output DVE ops only; dual-output accum ops wedge the device)
        scol = fin_pool.tile([2, 1], F32, name="scol", tag="scol")
        nc.vector.tensor_reduce(
            scol[:], psum_qsw[:], axis=mybir.AxisListType.X, op=OP.add,
        )
        Ctr = fin_pool.tile([2, C], F32, name="Ctr", tag="Ctr")
        nc.vector.tensor_scalar_mul(Ctr[:], psum_ew[:, 0:C], rden[:])

        scr = fin_pool.tile([2, C], F32, name="scr", tag="scr")
        nc.vector.tensor_mul(scr[:], Ctr[:], psum_qw[:, 0:C])
        dots1 = fin_pool.tile([2, 1], F32, name="dots1", tag="dots1")
        nc.vector.tensor_reduce(
            dots1[:], scr[:], axis=mybir.AxisListType.X, op=OP.add,
        )
        scr2 = fin_pool.tile([2, C], F32, name="scr2", tag="scr2")
        nc.vector.tensor_mul(scr2[:], Ctr[:], Ctr[:])
        dots2 = fin_pool.tile([2, 1], F32, name="dots2", tag="dots2")
        nc.vector.tensor_reduce(
            dots2[:], scr2[:], axis=mybir.AxisListType.X, op=OP.add,
        )

        t1 = fin_pool.tile([2, 1], F32, name="t1", tag="t1")
        nc.vector.tensor_mul(t1[:], dots2[:], ncol[:])
        t2 = fin_pool.tile([2, 1], F32, name="t2", tag="t2")
        nc.vector.scalar_tensor_tensor(
            out=t2[:], in0=dots1[:], scalar=-2.0, in1=scol[:],
            op0=OP.mult, op1=OP.add,
        )
        num = fin_pool.tile([2, 1], F32, name="num", tag="num")
        nc.vector.tensor_add(num[:], t1[:], t2[:])

        lss = fin_pool.tile([2, 1], F32, name="lss", tag="lss")
        nc.vector.tensor_mul(lss[:], num[:], rn[:])
        nc.vector.tensor_mul(lss[:], lss[:], gate[:])
        nc.vector.tensor_scalar_mul(lss[:], lss[:], 1.0 / C)
        nc.sync.dma_start(out=out2[:], in_=lss[:])

    nc.compile()
    return nc


def get_nc():
    if "nc" not in _CACHE:
        _CACHE["nc"] = _build()
    return _CACHE["nc"]


def _perm_mask(mask_b):
    """Downsampled mask permuted to the kernel's weight-column layout:
    Lm[p, 4i+g] = ds_flat[512i + 4p + g] (pure indexing)."""
    ds = mask_b[0, ::PATCH, ::PATCH].reshape(-1)           # (4096,)
    return np.ascontiguousarray(
        ds.reshape(N_DC, P, T).transpose(1, 0, 2).reshape(P, N_DC * T))


def make_in_maps(ep_mask_embed, ep_mask, query_mask_embed, query_mask):
    in_maps = []
    for b in range(B):
        in_maps.append({
            "ep_embed": np.ascontiguousarray(ep_mask_embed[b]),
            "q_embed": np.ascontiguousarray(query_mask_embed[b]),
            "lm": np.concatenate(
                [_perm_mask(ep_mask[b]), _perm_mask(query_mask[b])], axis=1),
        })
    return in_maps


def finalize(per_core):
    """per_core: list of 8 arrays [2,1] (pos;neg) -> full (3,) output."""
    vals = np.stack([np.asarray(r).reshape(2) for r in per_core])  # [8, 2]
    pos = vals[:, 0].astype(np.float64)
    neg = vals[:, 1].astype(np.float64)
    return np.array(
        [(pos + neg).mean(), pos.mean(), neg.mean()], dtype=np.float32
    )


def kernel(ep_mask_embed, ep_mask, query_mask_embed, query_mask):
    ep_mask_embed = np.asarray(ep_mask_embed, dtype=np.float32)
    ep_mask = np.asarray(ep_mask, dtype=np.float32)
    query_mask_embed = np.asarray(query_mask_embed, dtype=np.float32)
    query_mask = np.asarray(query_mask, dtype=np.float32)

    nc = get_nc()
    in_maps = make_in_maps(ep_mask_embed, ep_mask, query_mask_embed, query_mask)
    res = run_bass_kernel_spmd(nc, in_maps, list(range(B)))
    return finalize([r["out2"] for r in res.results])



# revision 3
# speedup vs baseline: 1.0255x; 1.0255x over previous
"""CenterLoss Trainium2 kernel.

Full inputs:
  ep_mask_embed    (8, 4096, 256) f32
  ep_mask          (8, 1, 1024, 1024) f32
  query_mask_embed (8, 4096, 256) f32
  query_mask       (8, 1, 1024, 1024) f32
Output: (3,) f32 = [mean(center_loss), mean(pos_loss), mean(neg_loss)]

Sharding: data-parallel, one batch sample per NeuronCore (8 cores).

Math (per sample, c=256, N=4096, m = mask downsampled to (N,)):
  PSUM-accumulated bf16 matmul chains (lhsT = [m, 1-m] mask columns,
  token-on-partition, contraction over 128 tokens per group):
    psum_ew [2,256] += [ep_m,1-ep_m]^T @ ep_embed
    psum_q  [2,512] += [q_m, 1-q_m]^T @ [q_embed | q_embed^2]   (fused N=512)
  epilogue (s = rowsum(psum_q[:,256:512]), qw = psum_q[:,0:256]):
    Ctr = epw/(n_ep+0.1); num = s - 2*dot(Ctr,qw) + n_q*dot(Ctr,Ctr)
    loss = num / (max(n_q,1)*c) * min(n_q,1)   per [pos;neg] partition row.

Performance structure (per-core stream is HBM-bound at ~410 GB/s, 8.4MB
=> ~20.5us floor):
  - t=32 token staging: partition p holds tokens 32p..32p+31, so every
    512KB chunk DMA is 128 descriptors x 4KB contiguous; mask weight
    columns are a plain host-side reshape.
  - ALL stream DMAs are emitted first on the sync queue (lm, ep0..7,
    q0..6, 4x128KB sub-chunks of the last q chunk) so HWDGE descriptor
    generation runs ahead and the SDMA backlog keeps the stream at line
    rate from the start; the last chunk is split so the tail pipeline
    (cast -> square -> matmul -> epilogue) is short.
  - ep streams entirely before q: the ep-center epilogue terms (Ctr,
    dot(Ctr,Ctr), n-scalars) are computed mid-stream, leaving a ~6-op
    DVE chain after the final matmul.
  - casts are split DVE/ACT; q^2 is computed on ACT (Square from f32).
  - two gated dummy matmuls per ep chunk keep the PE HAM activity
    monitor busy so it unthrottles (1.2 -> 2.4 GHz) before the q half,
    where 4 fused N=512 matmuls must chase each 512KB chunk.
The host only shards, downsamples (stride-16 indexing), and reshapes
the mask values per sample, and means the 8 per-core [pos, neg] pairs.
"""

import numpy as np
from contextlib import ExitStack

import concourse.bass as bass
import concourse.bacc as bacc
import concourse.tile as tile
from concourse import mybir
from concourse.bass_utils import run_bass_kernel_spmd

F32 = mybir.dt.float32
BF16 = mybir.dt.bfloat16

P = 128          # partitions
N_TOK = 4096     # tokens per sample (64*64 patches)
C = 256          # channels
T = 32           # tokens per partition (whole tensor)
NG = T           # 32 token groups of 128 tokens
GPC = 4          # groups per full 512KB chunk
N_EP_CH = 8      # ep chunks (all full)
N_Q_CH = 7       # full q chunks; last 4 groups stream as singles
B = 8            # batch == n cores
PATCH = 16

_CACHE = {}


def _build():
    """Build the per-core Bass program (identical on all cores)."""
    nc = bacc.Bacc("TRN2", target_bir_lowering=False, debug=False)

    ep_embed = nc.dram_tensor("ep_embed", [N_TOK, C], F32, kind="ExternalInput").ap()
    q_embed = nc.dram_tensor("q_embed", [N_TOK, C], F32, kind="ExternalInput").ap()
    # downsampled mask values in weight-column layout:
    # lm[p, g] = mask_ds[32p + g]; cols 0:32 = ep mask, cols 32:64 = q mask
    lm = nc.dram_tensor("lm", [P, 2 * NG], F32, kind="ExternalInput").ap()
    # [pos_loss; neg_loss] on partitions 0/1
    out2 = nc.dram_tensor("out2", [2, 1], F32, kind="ExternalOutput").ap()

    AF = mybir.ActivationFunctionType
    OP = mybir.AluOpType

    ep_src = ep_embed.rearrange("(p t) c -> p (t c)", t=T)   # [128, 32*256]
    q_src = q_embed.rearrange("(p t) c -> p (t c)", t=T)

    with tile.TileContext(nc) as tc, ExitStack() as ctx:
        const_pool = ctx.enter_context(tc.tile_pool(name="const", bufs=1))
        ep_f = ctx.enter_context(tc.tile_pool(name="ep_f", bufs=N_EP_CH))
        ep_b = ctx.enter_context(tc.tile_pool(name="ep_b", bufs=N_EP_CH))
        q_f = ctx.enter_context(tc.tile_pool(name="q_f", bufs=N_Q_CH))
        q_b = ctx.enter_context(tc.tile_pool(name="q_b", bufs=N_Q_CH))
        qs_f = ctx.enter_context(tc.tile_pool(name="qs_f", bufs=GPC))
        qs_b = ctx.enter_context(tc.tile_pool(name="qs_b", bufs=GPC))
        psum_pool = ctx.enter_context(
            tc.tile_pool(name="psum", bufs=1, space=bass.MemorySpace.PSUM)
        )
        fin_pool = ctx.enter_context(tc.tile_pool(name="fin", bufs=1))

        # ---- ALL stream DMAs first: they sit on the sync HWDGE queue in
        # this order and drain back-to-back at line rate ----
        lm_t = const_pool.tile([P, 2 * NG], F32, name="lm_t", tag="lm_t")
        nc.sync.dma_start(out=lm_t[:], in_=lm[:])

        te = []
        for j in range(N_EP_CH):
            t_ = ep_f.tile([P, GPC * C], F32, name=f"te{j}", tag="te")
            nc.sync.dma_start(
                out=t_[:], in_=ep_src[:, j * GPC * C:(j + 1) * GPC * C])
            te.append(t_)
        tq = []
        for j in range(N_Q_CH):
            t_ = q_f.tile([P, GPC * C], F32, name=f"tq{j}", tag="tq")
            nc.sync.dma_start(
                out=t_[:], in_=q_src[:, j * GPC * C:(j + 1) * GPC * C])
            tq.append(t_)
        tqs = []
        for s in range(GPC):
            g = N_Q_CH * GPC + s
            t_ = qs_f.tile([P, C], F32, name=f"tqs{s}", tag="tqs")
            nc.sync.dma_start(out=t_[:], in_=q_src[:, g * C:(g + 1) * C])
            tqs.append(t_)

        # ---- mask prep: per tensor L64 [128, 64] = [m (32) | 1-m (32)];
        # lhsT for group g = cols {g, g+32} (free stride 32) ----
        L = {}
        for li, nm in enumerate(("ep", "q")):
            L64 = const_pool.tile([P, 2 * NG], F32, name=f"L64_{nm}", tag=f"L64_{nm}")
            nc.vector.tensor_copy(L64[:, 0:NG], lm_t[:, li * NG:(li + 1) * NG])
            nc.vector.tensor_scalar(
                out=L64[:, NG:2 * NG], in0=L64[:, 0:NG], scalar1=-1.0,
                scalar2=1.0, op0=OP.mult, op1=OP.add,
            )
            Lb = const_pool.tile([P, 2 * NG], BF16, name=f"Lb_{nm}", tag=f"Lb_{nm}")
            nc.vector.tensor_copy(Lb[:], L64[:])
            L[nm] = Lb
            # per-partition mask sums -> [pos, neg] counts via a tiny matmul
            rs = const_pool.tile([P, 2], F32, name=f"rs_{nm}", tag=f"rs_{nm}")
            nc.vector.tensor_reduce(
                rs[:, 0:1], L64[:, 0:NG], axis=mybir.AxisListType.X, op=OP.add)
            nc.vector.tensor_reduce(
                rs[:, 1:2], L64[:, NG:2 * NG], axis=mybir.AxisListType.X,
                op=OP.add)
            L[nm + "_rs"] = rs

        ones1 = const_pool.tile([P, 1], F32, name="ones1", tag="ones1")
        nc.vector.memset(ones1[:], 1.0)

        def lhsT(nm, g):
            # 2-column AP [m, 1-m] with free stride NG
            return L[nm].rearrange("p (h c) -> p c h", h=2)[:, g, :]

        # PSUM accumulators (pos on partition 0, neg on partition 1):
        psum_ew = psum_pool.tile([2, C], F32, name="psum_ew", tag="pew")
        psum_q = psum_pool.tile([2, 2 * C], F32, name="psum_q", tag="pq")
        psum_en = psum_pool.tile([2, 1], F32, name="psum_en", tag="pen")
        psum_qn = psum_pool.tile([2, 1], F32, name="psum_qn", tag="pqn")
        psum_dum = psum_pool.tile([2, 2 * C], F32, name="psum_dum", tag="pdum")
        nc.tensor.matmul(psum_en[:], L["ep_rs"][:], ones1[:])
        nc.tensor.matmul(psum_qn[:], L["q_rs"][:], ones1[:])

        # count-derived epilogue scalars, hidden under the stream
        ncol = fin_pool.tile([2, 1], F32, name="ncol", tag="ncol")
        nc.vector.tensor_copy(ncol[:], psum_qn[:, 0:1])
        den = fin_pool.tile([2, 1], F32, name="den", tag="den")
        nc.vector.tensor_scalar_add(den[:], psum_en[:, 0:1], 0.1)
        rden = fin_pool.tile([2, 1], F32, name="rden", tag="rden")
        nc.vector.reciprocal(rden[:], den[:])
        nmax = fin_pool.tile([2, 1], F32, name="nmax", tag="nmax")
        nc.vector.tensor_scalar_max(nmax[:], ncol[:], 1.0)
        rn = fin_pool.tile([2, 1], F32, name="rn", tag="rn")
        nc.vector.reciprocal(rn[:], nmax[:])
        gate = fin_pool.tile([2, 1], F32, name="gate", tag="gate")
        nc.vector.tensor_scalar_min(gate[:], ncol[:], 1.0)
        sc = fin_pool.tile([2, 1], F32, name="sc", tag="sc")
        nc.vector.tensor_mul(sc[:], rn[:], gate[:])
        nc.vector.tensor_scalar_mul(sc[:], sc[:], 1.0 / C)

        # ---- ep half: cast split DVE/ACT, 4 matmuls + 2 PE-warming
        # dummies per chunk ----
        H = GPC * C // 2
        for j in range(N_EP_CH):
            rb = ep_b.tile([P, GPC * C], BF16, name=f"re{j}", tag="re")
            nc.vector.tensor_copy(rb[:, 0:H], te[j][:, 0:H])
            nc.scalar.copy(rb[:, H:2 * H], te[j][:, H:2 * H])
            for g in range(GPC):
                nc.tensor.matmul(
                    psum_ew[:], lhsT("ep", j * GPC + g),
                    rb[:, g * C:(g + 1) * C],
                    start=(j == 0 and g == 0),
                    stop=(j == N_EP_CH - 1 and g == GPC - 1),
                )
            # HAM warmers: keep PE busy through the ep half so it runs
            # unthrottled when the q half needs full matmul rate
            nc.tensor.matmul(psum_dum[:], lhsT("ep", 0), rb[:, 0:H],
                             start=True, stop=True)
            nc.tensor.matmul(psum_dum[:], lhsT("ep", 0), rb[:, H:2 * H],
                             start=True, stop=True)

        # ---- mid-stream epilogue precompute (ep-center terms) ----
        Ctr = fin_pool.tile([2, C], F32, name="Ctr", tag="Ctr")
        nc.vector.tensor_scalar_mul(Ctr[:], psum_ew[:, 0:C], rden[:])
        scr2 = fin_pool.tile([2, C], F32, name="scr2", tag="scr2")
        nc.vector.tensor_mul(scr2[:], Ctr[:], Ctr[:])
        dots2 = fin_pool.tile([2, 1], F32, name="dots2", tag="dots2")
        nc.vector.tensor_reduce(
            dots2[:], scr2[:], axis=mybir.AxisListType.X, op=OP.add)
        t1 = fin_pool.tile([2, 1], F32, name="t1", tag="t1")
        nc.vector.tensor_mul(t1[:], dots2[:], ncol[:])

        # ---- q half: DVE casts q, ACT squares (f32 in, bf16 out);
        # fused [q | q^2] N=512 matmuls ----
        def q_rhs(qb, g):
            # [128, 2, 256] AP: blocks {q bf16, q^2 bf16} for group g,
            # streamed as 512 free-dim elements matching psum_q columns
            return qb.rearrange("p (h gc) -> p h gc", h=2)[
                :, :, g * C:(g + 1) * C]

        for j in range(N_Q_CH):
            qb = q_b.tile([P, 2 * GPC * C], BF16, name=f"qb{j}", tag="qb")
            nc.vector.tensor_copy(qb[:, 0:GPC * C], tq[j][:])
            nc.scalar.activation(
                out=qb[:, GPC * C:2 * GPC * C], in_=tq[j][:], func=AF.Square)
            for g in range(GPC):
                idx = j * GPC + g
                nc.tensor.matmul(
                    psum_q[:], lhsT("q", idx), q_rhs(qb, g),
                    start=(idx == 0), stop=False,
                )
        for s in range(GPC):
            idx = N_Q_CH * GPC + s
            qb = qs_b.tile([P, 2 * C], BF16, name=f"qsb{s}", tag="qsb")
            nc.vector.tensor_copy(qb[:, 0:C], tqs[s][:])
            nc.scalar.activation(out=qb[:, C:2 * C], in_=tqs[s][:], func=AF.Square)
            nc.tensor.matmul(
                psum_q[:], lhsT("q", idx), qb[:],
                start=False, stop=(s == GPC - 1),
            )

        # ---- final epilogue: short DVE chain after the last matmul ----
        scr = fin_pool.tile([2, C], F32, name="scr", tag="scr")
        nc.vector.tensor_mul(scr[:], Ctr[:], psum_q[:, 0:C])
        dots1 = fin_pool.tile([2, 1], F32, name="dots1", tag="dots1")
        nc.vector.tensor_reduce(
            dots1[:], scr[:], axis=mybir.AxisListType.X, op=OP.add)
        scol = fin_pool.tile([2, 1], F32, name="scol", tag="scol")
        nc.vector.tensor_reduce(
            scol[:], psum_q[:, C:2 * C], axis=mybir.AxisListType.X, op=OP.add)
        t2 = fin_pool.tile([2, 1], F32, name="t2", tag="t2")
        nc.vector.scalar_tensor_tensor(
            out=t2[:], in0=dots1[:], scalar=-2.0, in1=scol[:],
            op0=OP.mult, op1=OP.add,
        )
        num = fin_pool.tile([2, 1], F32, name="num", tag="num")
        nc.vector.tensor_add(num[:], t2[:], t1[:])
        lss = fin_pool.tile([2, 1], F32, name="lss", tag="lss")
        nc.vector.tensor_mul(lss[:], num[:], sc[:])
        nc.sync.dma_start(out=out2[:], in_=lss[:])

    nc.compile()
    return nc


def get_nc():
    if "nc" not in _CACHE:
        _CACHE["nc"] = _build()
    return _CACHE["nc"]


def _perm_mask(mask_b):
    """Downsampled mask in the kernel's weight-column layout:
    Lm[p, g] = ds_flat[32p + g] (plain reshape)."""
    ds = mask_b[0, ::PATCH, ::PATCH].reshape(-1)           # (4096,)
    return np.ascontiguousarray(ds.reshape(P, T))


def make_in_maps(ep_mask_embed, ep_mask, query_mask_embed, query_mask):
    in_maps = []
    for b in range(B):
        in_maps.append({
            "ep_embed": np.ascontiguousarray(ep_mask_embed[b]),
            "q_embed": np.ascontiguousarray(query_mask_embed[b]),
            "lm": np.concatenate(
                [_perm_mask(ep_mask[b]), _perm_mask(query_mask[b])], axis=1),
        })
    return in_maps


def finalize(per_core):
    """per_core: list of 8 arrays [2,1] (pos;neg) -> full (3,) output."""
    vals = np.stack([np.asarray(r).reshape(2) for r in per_core])  # [8, 2]
    pos = vals[:, 0].astype(np.float64)
    neg = vals[:, 1].astype(np.float64)
    return np.array(
        [(pos + neg).mean(), pos.mean(), neg.mean()], dtype=np.float32
    )


def kernel(ep_mask_embed, ep_mask, query_mask_embed, query_mask):
    ep_mask_embed = np.asarray(ep_mask_embed, dtype=np.float32)
    ep_mask = np.asarray(ep_mask, dtype=np.float32)
    query_mask_embed = np.asarray(query_mask_embed, dtype=np.float32)
    query_mask = np.asarray(query_mask, dtype=np.float32)

    nc = get_nc()
    in_maps = make_in_maps(ep_mask_embed, ep_mask, query_mask_embed, query_mask)
    res = run_bass_kernel_spmd(nc, in_maps, list(range(B)))
    return finalize([r["out2"] for r in res.results])


# revision 6
# speedup vs baseline: 1.0337x; 1.0080x over previous
"""CenterLoss Trainium2 kernel.

Full inputs:
  ep_mask_embed    (8, 4096, 256) f32
  ep_mask          (8, 1, 1024, 1024) f32
  query_mask_embed (8, 4096, 256) f32
  query_mask       (8, 1, 1024, 1024) f32
Output: (3,) f32 = [mean(center_loss), mean(pos_loss), mean(neg_loss)]

Sharding: data-parallel, one batch sample per NeuronCore (8 cores).

Math (per sample, c=256, N=4096, m = mask downsampled to (N,)):
  PSUM-accumulated bf16 matmul chains (lhsT = [m, 1-m] mask columns,
  token-on-partition, contraction over 128 tokens per group):
    psum_ew [2,256] += [ep_m,1-ep_m]^T @ ep_embed
    psum_q  [2,512] += [q_m, 1-q_m]^T @ [q_embed | q_embed^2]   (fused N=512)
  epilogue (scol = rowsum(qsw), qw/qsw = psum_q halves):
    Ctr = epw/(n_ep+0.1)
    num = scol + rowsum([-2*qw | n_q*Ctr] . [Ctr | Ctr])
        = scol - 2*dot(Ctr,qw) + n_q*dot(Ctr,Ctr)
    loss = num / (max(n_q,1)*c) * min(n_q,1)   per [pos;neg] partition row.

Performance structure (per-core stream is HBM-bound at ~410 GB/s, 8.4MB
=> ~20.5us floor; measured preamble ~6.6us and post-output ~4us are
framework-fixed):
  - t=32 token staging: partition p holds tokens 32p..32p+31, so every
    512KB chunk DMA is 128 descriptors x 4KB contiguous; mask weight
    columns are a plain host-side reshape.
  - ALL stream DMAs are emitted first on the sync HWDGE queue
    (lm, q0..7, ep0..7) so descriptor generation runs ahead and the
    SDMA backlog holds the stream at line rate.
  - q streams FIRST: its heavy chase (DVE cast + ACT/DVE squares +
    4 fused N=512 matmuls per 512KB chunk) hides under the stream
    middle, and the ep tail work after the last chunk lands is tiny
    (two half-casts + 4 N=256 matmuls + a ~8-op DVE epilogue).
  - each matmul chain accumulates into TWO alternating PSUM banks:
    back-to-back matmuls into one bank serialize at full fill+drain
    latency ((398+N)/2.4 ns); alternating banks restores the ~N/2.4
    streaming rate.  Banks are merged with one DVE add off the
    critical path (q) / on a short tail (ep).
  - all count-derived scalars and the q-side epilogue terms (qw merge,
    -2*qw, rowsum(qsw)) are precomputed mid-stream.
The host only shards, downsamples (stride-16 indexing), and reshapes
the mask values per sample, and means the 8 per-core [pos, neg] pairs.
"""

import numpy as np
from contextlib import ExitStack

import concourse.bass as bass
import concourse.bacc as bacc
import concourse.tile as tile
from concourse import mybir
from concourse.bass_utils import run_bass_kernel_spmd

F32 = mybir.dt.float32
BF16 = mybir.dt.bfloat16

P = 128          # partitions
N_TOK = 4096     # tokens per sample (64*64 patches)
C = 256          # channels
T = 32           # tokens per partition (whole tensor)
NG = T           # 32 token groups of 128 tokens
GPC = 4          # groups per 512KB chunk
N_CH = 8         # chunks per tensor
B = 8            # batch == n cores
PATCH = 16

_CACHE = {}


def _build():
    """Build the per-core Bass program (identical on all cores)."""
    nc = bacc.Bacc("TRN2", target_bir_lowering=False, debug=False)

    ep_embed = nc.dram_tensor("ep_embed", [N_TOK, C], F32, kind="ExternalInput").ap()
    q_embed = nc.dram_tensor("q_embed", [N_TOK, C], F32, kind="ExternalInput").ap()
    # downsampled mask values in weight-column layout:
    # lm[p, g] = mask_ds[32p + g]; cols 0:32 = ep mask, cols 32:64 = q mask
    lm = nc.dram_tensor("lm", [P, 2 * NG], F32, kind="ExternalInput").ap()
    # [pos_loss; neg_loss] on partitions 0/1
    out2 = nc.dram_tensor("out2", [2, 1], F32, kind="ExternalOutput").ap()

    AF = mybir.ActivationFunctionType
    OP = mybir.AluOpType

    ep_src = ep_embed.rearrange("(p t) c -> p (t c)", t=T)   # [128, 32*256]
    q_src = q_embed.rearrange("(p t) c -> p (t c)", t=T)
    W = GPC * C                                              # 1024 f32 / chunk

    with tile.TileContext(nc) as tc, ExitStack() as ctx:
        const_pool = ctx.enter_context(tc.tile_pool(name="const", bufs=1))
        q_f = ctx.enter_context(tc.tile_pool(name="q_f", bufs=N_CH))
        q_b = ctx.enter_context(tc.tile_pool(name="q_b", bufs=N_CH))
        ep_f = ctx.enter_context(tc.tile_pool(name="ep_f", bufs=N_CH))
        ep_b = ctx.enter_context(tc.tile_pool(name="ep_b", bufs=N_CH))
        psum_pool = ctx.enter_context(
            tc.tile_pool(name="psum", bufs=1, space=bass.MemorySpace.PSUM)
        )
        fin_pool = ctx.enter_context(tc.tile_pool(name="fin", bufs=1))

        # ---- ALL stream DMAs first: they sit on the sync HWDGE queue in
        # this order and drain back-to-back at line rate ----
        lm_t = const_pool.tile([P, 2 * NG], F32, name="lm_t", tag="lm_t")
        nc.sync.dma_start(out=lm_t[:], in_=lm[:])

        tq = []
        for j in range(N_CH):
            t_ = q_f.tile([P, W], F32, name=f"tq{j}", tag="tq")
            nc.sync.dma_start(out=t_[:], in_=q_src[:, j * W:(j + 1) * W])
            tq.append(t_)
        te = []
        for j in range(N_CH):
            t_ = ep_f.tile([P, W], F32, name=f"te{j}", tag="te")
            nc.sync.dma_start(out=t_[:], in_=ep_src[:, j * W:(j + 1) * W])
            te.append(t_)

        # ---- mask prep: per tensor L64 [128, 64] = [m (32) | 1-m (32)];
        # lhsT for group g = cols {g, g+32} (free stride 32) ----
        L = {}
        for li, nm in enumerate(("ep", "q")):
            L64 = const_pool.tile([P, 2 * NG], F32, name=f"L64_{nm}", tag=f"L64_{nm}")
            nc.vector.tensor_copy(L64[:, 0:NG], lm_t[:, li * NG:(li + 1) * NG])
            nc.vector.tensor_scalar(
                out=L64[:, NG:2 * NG], in0=L64[:, 0:NG], scalar1=-1.0,
                scalar2=1.0, op0=OP.mult, op1=OP.add,
            )
            Lb = const_pool.tile([P, 2 * NG], BF16, name=f"Lb_{nm}", tag=f"Lb_{nm}")
            nc.vector.tensor_copy(Lb[:], L64[:])
            L[nm] = Lb
            # per-partition mask sums -> [pos, neg] counts via a tiny matmul
            rs = const_pool.tile([P, 2], F32, name=f"rs_{nm}", tag=f"rs_{nm}")
            nc.vector.tensor_reduce(
                rs[:, 0:1], L64[:, 0:NG], axis=mybir.AxisListType.X, op=OP.add)
            nc.vector.tensor_reduce(
                rs[:, 1:2], L64[:, NG:2 * NG], axis=mybir.AxisListType.X,
                op=OP.add)
            L[nm + "_rs"] = rs

        ones1 = const_pool.tile([P, 1], F32, name="ones1", tag="ones1")
        nc.vector.memset(ones1[:], 1.0)

        def lhsT(nm, g):
            # 2-column AP [m, 1-m] with free stride NG
            return L[nm].rearrange("p (h c) -> p c h", h=2)[:, g, :]

        # PSUM accumulators, two banks per chain (pos=partition 0, neg=1).
        # Full-bank [2, 512] allocations keep the two banks of a chain in
        # physically distinct PSUM banks so matmuls pipeline.
        psum_q = [
            psum_pool.tile([2, 2 * C], F32, name=f"psum_q{b_}", tag=f"pq{b_}")
            for b_ in range(2)
        ]
        psum_ew = [
            psum_pool.tile([2, 2 * C], F32, name=f"psum_ew{b_}", tag=f"pew{b_}")
            for b_ in range(2)
        ]
        psum_en = psum_pool.tile([2, 1], F32, name="psum_en", tag="pen")
        psum_qn = psum_pool.tile([2, 1], F32, name="psum_qn", tag="pqn")
        nc.tensor.matmul(psum_en[:], L["ep_rs"][:], ones1[:])
        nc.tensor.matmul(psum_qn[:], L["q_rs"][:], ones1[:])

        # count-derived epilogue scalars, hidden under the stream
        ncol = fin_pool.tile([2, 1], F32, name="ncol", tag="ncol")
        nc.vector.tensor_copy(ncol[:], psum_qn[:, 0:1])
        den = fin_pool.tile([2, 1], F32, name="den", tag="den")
        nc.vector.tensor_scalar_add(den[:], psum_en[:, 0:1], 0.1)
        rden = fin_pool.tile([2, 1], F32, name="rden", tag="rden")
        nc.vector.reciprocal(rden[:], den[:])
        nmax = fin_pool.tile([2, 1], F32, name="nmax", tag="nmax")
        nc.vector.tensor_scalar_max(nmax[:], ncol[:], 1.0)
        rn = fin_pool.tile([2, 1], F32, name="rn", tag="rn")
        nc.vector.reciprocal(rn[:], nmax[:])
        gate = fin_pool.tile([2, 1], F32, name="gate", tag="gate")
        nc.vector.tensor_scalar_min(gate[:], ncol[:], 1.0)
        sc = fin_pool.tile([2, 1], F32, name="sc", tag="sc")
        nc.vector.tensor_mul(sc[:], rn[:], gate[:])
        nc.vector.tensor_scalar_mul(sc[:], sc[:], 1.0 / C)

        # ---- q half (first): DVE casts q, squares split ACT (768 from
        # f32) / DVE (256 from bf16); fused [q | q^2] N=512 matmuls ----
        def q_rhs(qb, g):
            # [128, 2, 256] AP: blocks {q bf16, q^2 bf16} for group g,
            # streamed as 512 free-dim elements matching psum_q columns
            return qb.rearrange("p (h gc) -> p h gc", h=2)[
                :, :, g * C:(g + 1) * C]

        SQH = 768
        for j in range(N_CH):
            qb = q_b.tile([P, 2 * W], BF16, name=f"qb{j}", tag="qb")
            nc.vector.tensor_copy(qb[:, 0:W], tq[j][:])
            nc.scalar.activation(
                out=qb[:, W:W + SQH], in_=tq[j][:, 0:SQH], func=AF.Square)
            nc.vector.tensor_mul(
                qb[:, W + SQH:2 * W], qb[:, SQH:W], qb[:, SQH:W])
            for g in range(GPC):
                idx = j * GPC + g
                nc.tensor.matmul(
                    psum_q[idx % 2][:], lhsT("q", idx), q_rhs(qb, g),
                    start=(idx < 2), stop=(idx >= NG - 2),
                )

        # ---- mid-stream q-side epilogue precompute ----
        qw_s = fin_pool.tile([2, 2 * C], F32, name="qw_s", tag="qw_s")
        nc.vector.tensor_copy(qw_s[:], psum_q[0][:])
        nc.vector.tensor_add(qw_s[:], qw_s[:], psum_q[1][:])
        w0 = fin_pool.tile([2, C], F32, name="w0", tag="w0")
        nc.vector.tensor_scalar_mul(w0[:], qw_s[:, 0:C], -2.0)
        scol = fin_pool.tile([2, 1], F32, name="scol", tag="scol")
        nc.vector.tensor_reduce(
            scol[:], qw_s[:, C:2 * C], axis=mybir.AxisListType.X, op=OP.add)

        # ---- ep half (second): cast split DVE/ACT, 4 N=256 matmuls ----
        H = W // 2
        for j in range(N_CH):
            rb = ep_b.tile([P, W], BF16, name=f"re{j}", tag="re")
            nc.vector.tensor_copy(rb[:, 0:H], te[j][:, 0:H])
            nc.scalar.copy(rb[:, H:W], te[j][:, H:W])
            for g in range(GPC):
                idx = j * GPC + g
                nc.tensor.matmul(
                    psum_ew[idx % 2][:, 0:C], lhsT("ep", idx),
                    rb[:, g * C:(g + 1) * C],
                    start=(idx < 2), stop=(idx >= NG - 2),
                )

        # ---- tail epilogue: short DVE chain after the last matmul ----
        eww = fin_pool.tile([2, C], F32, name="eww", tag="eww")
        nc.vector.tensor_copy(eww[:], psum_ew[0][:, 0:C])
        nc.vector.tensor_add(eww[:], eww[:], psum_ew[1][:, 0:C])
        Ctr = fin_pool.tile([2, C], F32, name="Ctr", tag="Ctr")
        nc.vector.tensor_scalar_mul(Ctr[:], eww[:], rden[:])
        w1 = fin_pool.tile([2, C], F32, name="w1", tag="w1")
        nc.vector.tensor_scalar_mul(w1[:], Ctr[:], ncol[:])
        v = fin_pool.tile([2, 2 * C], F32, name="v", tag="v")
        nc.vector.tensor_mul(v[:, 0:C], w0[:], Ctr[:])
        nc.vector.tensor_mul(v[:, C:2 * C], w1[:], Ctr[:])
        vr = fin_pool.tile([2, 1], F32, name="vr", tag="vr")
        nc.vector.tensor_reduce(
            vr[:], v[:], axis=mybir.AxisListType.X, op=OP.add)
        num = fin_pool.tile([2, 1], F32, name="num", tag="num")
        nc.vector.tensor_add(num[:], vr[:], scol[:])
        lss = fin_pool.tile([2, 1], F32, name="lss", tag="lss")
        nc.vector.tensor_mul(lss[:], num[:], sc[:])
        nc.sync.dma_start(out=out2[:], in_=lss[:])

    nc.compile()
    return nc


def get_nc():
    if "nc" not in _CACHE:
        _CACHE["nc"] = _build()
    return _CACHE["nc"]


def _perm_mask(mask_b):
    """Downsampled mask in the kernel's weight-column layout:
    Lm[p, g] = ds_flat[32p + g] (plain reshape)."""
    ds = mask_b[0, ::PATCH, ::PATCH].reshape(-1)           # (4096,)
    return np.ascontiguousarray(ds.reshape(P, T))


def make_in_maps(ep_mask_embed, ep_mask, query_mask_embed, query_mask):
    in_maps = []
    for b in range(B):
        in_maps.append({
            "ep_embed": np.ascontiguousarray(ep_mask_embed[b]),
            "q_embed": np.ascontiguousarray(query_mask_embed[b]),
            "lm": np.concatenate(
                [_perm_mask(ep_mask[b]), _perm_mask(query_mask[b])], axis=1),
        })
    return in_maps


def finalize(per_core):
    """per_core: list of 8 arrays [2,1] (pos;neg) -> full (3,) output."""
    vals = np.stack([np.asarray(r).reshape(2) for r in per_core])  # [8, 2]
    pos = vals[:, 0].astype(np.float64)
    neg = vals[:, 1].astype(np.float64)
    return np.array(
        [(pos + neg).mean(), pos.mean(), neg.mean()], dtype=np.float32
    )


def kernel(ep_mask_embed, ep_mask, query_mask_embed, query_mask):
    ep_mask_embed = np.asarray(ep_mask_embed, dtype=np.float32)
    ep_mask = np.asarray(ep_mask, dtype=np.float32)
    query_mask_embed = np.asarray(query_mask_embed, dtype=np.float32)
    query_mask = np.asarray(query_mask, dtype=np.float32)

    nc = get_nc()
    in_maps = make_in_maps(ep_mask_embed, ep_mask, query_mask_embed, query_mask)
    res = run_bass_kernel_spmd(nc, in_maps, list(range(B)))
    return finalize([r["out2"] for r in res.results])
